# revision 28
# baseline (speedup 1.0000x reference)
"""Trainium2 Bass kernel for nn_BoxDetectionLoss (8-core data parallel).

Math: reference loss = sum_{a,r,c}[ has_match ? coord+conf_loss : conf^2 ] / denom.
A pixel (r,c) can only match a target box t if r==tb[t,0] and c==tb[t,1]
(T=16 boxes per image), so the dense term is just sum sigmoid(conf_ch)^2 over
channels {2,5,8}; the match term is a correction at <=16 pixels x 3 anchors,
computed from 144 gathered elements per image.

Each of the 8 cores handles one batch image:
  - dense: 6 chunk DMAs of [128,1024] f32 spread over the three DMA rings
    (sync/scalar HWDGE + gpsimd SWDGE) so compute pipelines with the loads;
    the shared-tag bufs=4 pool throttles DMAs in flight, which staggers ring
    contention. Per chunk: ACT sigmoid (bf16 out); the square+reduce runs as
    DVE bf16 tensor_tensor mult + PE matmul against a ones vector that
    accumulates column sums into one [1,512] PSUM bank (the idle PE replaces
    DVE's slow 1x-mode tensor_reduce); the last-arriving chunk instead uses
    ACT Square w/ accum_out so the final PSUM fold stays off the tail.
  - correction: host precomputes all tb-derived constants (r1,c1,r2,c2,tp,
    keep-mask, gather offsets) into one packed [16,15] i32 tensor; device
    gathers pol at the 16 box pixels x 9 channels, applies sigmoid, and ~19
    tiny [16,3] DVE ops produce the correction column - all overlapped with
    the dense DMA window.
  - output: ACC [128, 7] partials DMA'd out; host sums and divides by denom.

Only standard-ISA engine ops are used: custom DVE ops (tensor_tensor_reduce
etc.) crash this runtime's exec units (no custom ucode tables loaded).
"""

import numpy as np

B, C, H, W = 8, 9, 512, 512
T = 16
N_CORES = 8
CONF_CH = (2, 5, 8)
DENOM = float(B * H * W * 3)
MAGIC = 12582912.0  # 1.5 * 2^23: x+MAGIC-MAGIC rounds to nearest-even int
NCHUNK = 6
NCOL = NCHUNK + 1  # dense chunk columns + correction column

# Dense chunk schedule: channel, column window of the [128,2048] channel view,
# DMA queue, square engine (act=True -> ACT Square+accum, else DVE TT+TR).
# One channel per queue, split big-first/small-second: the big first wave
# starts compute early; the small second wave keeps the post-DMA tail short.
# Asymmetric sizes stagger each ring's mid-stream sem-inc (completion
# receipt) so the ~2us stalls of different rings never align and can hide
# behind the other rings' transfers.
CHUNK_SPEC = (
    dict(ch=2, lo=0, n=768, q="sync", act=False),       # c0
    dict(ch=5, lo=0, n=1024, q="scalar", act=False),    # c1
    dict(ch=8, lo=0, n=1280, q="gpsimd", act=False),    # c2
    dict(ch=2, lo=768, n=1280, q="sync", act=False),    # c3
    dict(ch=5, lo=1024, n=1024, q="scalar", act=False),  # c4
    dict(ch=8, lo=1280, n=768, q="gpsimd", act=True),   # c5
)
ACT_ORDER = (0, 1, 2, 3, 4, 5)  # expected arrival order for the compute loop
DVE_SQ_ORDER = (0, 1, 2, 3)  # DVE-square chunks in expected arrival order

_PROG = None


def _emit_correction(nc, sp, ACC, bass, mybir, GS, CF, col):
    """~19 tiny [16,3] DVE ops: prediction, rounding, match test, loss terms.

    GS: [T, C] f32 sigmoid of gathered pol values at box pixels.
    CF: [T, 15] f32 view of packed constants; cols 0-5 = r1,c1,r2,c2,tp,keep.
    """
    f32 = mybir.dt.float32
    ALU = mybir.AluOpType
    R1, C1 = CF[:, 0:1], CF[:, 1:2]
    R2, C2 = CF[:, 2:3], CF[:, 3:4]
    TP, KEEP = CF[:, 4:5], CF[:, 5:6]

    # channel ch = 3a + k: k=0 delta_r, k=1 delta_c, k=2 conf
    gs3 = GS[:].rearrange("p (a k) -> p k a", k=3)

    predr = sp.tile([T, 3], f32)
    nc.vector.tensor_scalar(out=predr[:], in0=gs3[:, 0, :], scalar1=9.0,
                            scalar2=R1, op0=ALU.mult, op1=ALU.add)
    nc.vector.tensor_scalar(out=predr[:], in0=predr[:], scalar1=511.0,
                            scalar2=0.0, op0=ALU.min, op1=ALU.max)
    predc = sp.tile([T, 3], f32)
    nc.vector.tensor_scalar(out=predc[:], in0=gs3[:, 1, :], scalar1=16.0,
                            scalar2=C1, op0=ALU.mult, op1=ALU.add)
    nc.vector.tensor_scalar(out=predc[:], in0=predc[:], scalar1=511.0,
                            scalar2=0.0, op0=ALU.min, op1=ALU.max)

    # round-to-nearest-even via the +/- 1.5*2^23 trick, fused in one op
    rr = sp.tile([T, 3], f32)
    nc.vector.tensor_scalar(out=rr[:], in0=predr[:], scalar1=MAGIC,
                            scalar2=MAGIC, op0=ALU.add, op1=ALU.subtract)
    rc = sp.tile([T, 3], f32)
    nc.vector.tensor_scalar(out=rc[:], in0=predc[:], scalar1=MAGIC,
                            scalar2=MAGIC, op0=ALU.add, op1=ALU.subtract)

    # match mask; keep folds out duplicate boxes (first-match semantics)
    m = sp.tile([T, 3], f32)
    nc.vector.tensor_scalar(out=m[:], in0=rr[:], scalar1=R2, scalar2=None,
                            op0=ALU.is_equal)
    m2 = sp.tile([T, 3], f32)
    nc.vector.tensor_scalar(out=m2[:], in0=rc[:], scalar1=C2, scalar2=None,
                            op0=ALU.is_equal)
    nc.vector.tensor_tensor(out=m[:], in0=m[:], in1=m2[:], op=ALU.mult)
    nc.vector.tensor_scalar(out=m[:], in0=m[:], scalar1=KEEP, scalar2=None,
                            op0=ALU.mult)

    # contribution = |predr-r2| + |predc-c2| + tp*(tp-2*conf)
    # |x| as max(predr-r2, r2-predr): abs is not in the DVE TS ISA
    d1 = sp.tile([T, 3], f32)
    nc.vector.tensor_scalar(out=d1[:], in0=predr[:], scalar1=R2, scalar2=None,
                            op0=ALU.subtract)
    d1n = sp.tile([T, 3], f32)
    nc.vector.tensor_scalar(out=d1n[:], in0=predr[:], scalar1=-1.0, scalar2=R2,
                            op0=ALU.mult, op1=ALU.add)
    nc.vector.tensor_tensor(out=d1[:], in0=d1[:], in1=d1n[:], op=ALU.max)
    d2 = sp.tile([T, 3], f32)
    nc.vector.tensor_scalar(out=d2[:], in0=predc[:], scalar1=C2, scalar2=None,
                            op0=ALU.subtract)
    d2n = sp.tile([T, 3], f32)
    nc.vector.tensor_scalar(out=d2n[:], in0=predc[:], scalar1=-1.0, scalar2=C2,
                            op0=ALU.mult, op1=ALU.add)
    nc.vector.tensor_tensor(out=d2[:], in0=d2[:], in1=d2n[:], op=ALU.max)
    nc.vector.tensor_tensor(out=d1[:], in0=d1[:], in1=d2[:], op=ALU.add)
    cf = sp.tile([T, 3], f32)
    nc.vector.tensor_scalar(out=cf[:], in0=gs3[:, 2, :], scalar1=-2.0,
                            scalar2=TP, op0=ALU.mult, op1=ALU.add)
    nc.vector.tensor_scalar(out=cf[:], in0=cf[:], scalar1=TP, scalar2=None,
                            op0=ALU.mult)
    nc.vector.tensor_tensor(out=d1[:], in0=d1[:], in1=cf[:], op=ALU.add)

    # ACC[0:T, col] = sum_anchors m * d1
    nc.vector.tensor_tensor(out=m[:], in0=m[:], in1=d1[:], op=ALU.mult)
    nc.vector.tensor_reduce(out=ACC[0:T, col:col + 1], in_=m[:],
                            axis=mybir.AxisListType.X, op=ALU.add)


def _build_program(chunk_spec=CHUNK_SPEC, act_order=ACT_ORDER,
                   dve_sq_order=DVE_SQ_ORDER, corr=True, gather=True):
    import concourse.bass as bass
    import concourse.tile as tile
    from concourse import bacc, mybir

    f32 = mybir.dt.float32
    i32 = mybir.dt.int32
    bf16 = mybir.dt.bfloat16
    ALU = mybir.AluOpType
    ACT_F = mybir.ActivationFunctionType
    nchunk = len(chunk_spec)

    nc = bacc.Bacc(
        "TRN2", target_bir_lowering=False, debug=False, num_devices=N_CORES
    )
    pol = nc.dram_tensor("pol", [C, H, W], f32, kind="ExternalInput").ap()
    cst = nc.dram_tensor("cst", [T, 15], i32, kind="ExternalInput").ap()
    out = nc.dram_tensor("out", [128, nchunk + 1], f32,
                         kind="ExternalOutput").ap()

    with tile.TileContext(nc) as tc:
        with (
            tc.tile_pool(name="io", bufs=4) as io,
            tc.tile_pool(name="acc", bufs=1) as accp,
            tc.tile_pool(name="small", bufs=1) as sp,
            tc.tile_pool(name="psum", bufs=1, space="PSUM") as psp,
        ):
            ACC = accp.tile([128, nchunk + 1], f32)
            nc.vector.memset(ACC[:], 0.0)
            # ones vector: PE matmul against it sums SQ over partitions into
            # one accumulating PSUM bank (frees DVE from 1x-mode reduces)
            ONES = sp.tile([128, 1], bf16)
            nc.vector.memset(ONES[:], 1.0)
            PS = psp.tile([1, 512], f32, space="PSUM")

            # correction inputs: packed constants + indirect gather of the
            # 16 box pixels x 9 channels (offsets precomputed on host)
            CST = sp.tile([T, 15], i32)
            G = sp.tile([T, C], f32)
            if corr:
                nc.scalar.dma_start(CST[:], cst[:])
            maxn = max(cs["n"] for cs in chunk_spec)
            xts = []
            for k, cs in enumerate(chunk_spec):
                view = pol[cs["ch"]].rearrange("(p a) w -> p (a w)", p=128)
                Xfull = io.tile([128, maxn], f32, tag="in")
                Xt = Xfull[:, 0:cs["n"]]
                getattr(nc, cs["q"]).dma_start(
                    Xt, view[:, cs["lo"]:cs["lo"] + cs["n"]])
                xts.append(Xt)

            if corr and gather:
                # dense offset tile for the SWDGE offset walker
                OFFD = sp.tile([T, C], i32)
                nc.vector.tensor_copy(OFFD[:], CST[:, 6:15])
                nc.gpsimd.indirect_dma_start(
                    out=G[:], out_offset=None,
                    in_=pol.rearrange("c h (w a) -> (c h w) a", a=1),
                    in_offset=bass.IndirectOffsetOnAxis(ap=OFFD[:], axis=0),
                )
            elif corr:
                nc.vector.memset(G[:], 0.0)

            GS = sp.tile([T, C], f32)
            CF = CST[:].bitcast(f32)
            dve_ks = [k for k in act_order if not chunk_spec[k]["act"]]
            for k in act_order:
                cs, Xt = chunk_spec[k], xts[k]
                SIGfull = io.tile([128, maxn], bf16, tag="sig")
                SIG = SIGfull[:, 0:cs["n"]]
                nc.scalar.activation(SIG, Xt, ACT_F.Sigmoid)
                if corr and k == act_order[0]:
                    nc.scalar.activation(GS[:], G[:], ACT_F.Sigmoid)
                SQfull = io.tile([128, maxn], bf16, tag="sq")
                SQ = SQfull[:, 0:cs["n"]]
                if cs["act"]:
                    nc.scalar.activation(SQ, SIG, ACT_F.Square,
                                         accum_out=ACC[:, k:k + 1])
                else:
                    nc.vector.tensor_tensor(out=SQ, in0=SIG, in1=SIG,
                                            op=ALU.mult)
                    for blk in range(0, cs["n"], 512):
                        w = min(512, cs["n"] - blk)
                        nc.tensor.matmul(
                            out=PS[:, 0:w], lhsT=ONES[:],
                            rhs=SQ[:, blk:blk + w],
                            start=(k == dve_ks[0] and blk == 0),
                            stop=(k == dve_ks[-1] and blk + w >= cs["n"]),
                        )
                if corr and k == act_order[1]:
                    _emit_correction(nc, sp, ACC, bass, mybir, GS, CF, nchunk)

            # fold the PE-accumulated column sums into ACC[0,0]
            nc.vector.tensor_reduce(out=ACC[0:1, 0:1], in_=PS[:],
                                    axis=mybir.AxisListType.X, op=ALU.add)
            nc.sync.dma_start(out[:], ACC[:])

    nc.compile()
    return nc


def get_program():
    global _PROG
    if _PROG is None:
        _PROG = _build_program()
    return _PROG


def make_in_maps(policy_output, target_boxes, target_probs):
    policy_output = np.ascontiguousarray(np.asarray(policy_output, dtype=np.float32))
    target_boxes = np.ascontiguousarray(np.asarray(target_boxes, dtype=np.int32))
    target_probs = np.ascontiguousarray(np.asarray(target_probs, dtype=np.float32))
    assert policy_output.shape == (B, C, H, W)
    in_maps = []
    for i in range(N_CORES):
        tb = target_boxes[i].astype(np.int64)
        r1, c1, r2, c2 = tb[:, 0], tb[:, 1], tb[:, 2], tb[:, 3]
        off = (np.arange(C, dtype=np.int64)[None, :] * (H * W)
               + (r1 * W + c1)[:, None]).astype(np.int32)
        keep = np.ones(T, dtype=np.float32)
        seen = set()
        for t in range(T):
            key = (int(r1[t]), int(c1[t]), int(r2[t]), int(c2[t]))
            if key in seen:
                keep[t] = 0.0
            else:
                seen.add(key)
        cstf = np.zeros((T, 15), dtype=np.float32)
        cstf[:, 0] = r1
        cstf[:, 1] = c1
        cstf[:, 2] = r2
        cstf[:, 3] = c2
        cstf[:, 4] = target_probs[i]
        cstf[:, 5] = keep
        cst = cstf.view(np.int32).copy()
        cst[:, 6:15] = off
        in_maps.append({"pol": policy_output[i], "cst": cst})
    return in_maps


def kernel(policy_output, target_boxes, target_probs):
    from concourse.bass_utils import run_bass_kernel_spmd

    nc = get_program()
    in_maps = make_in_maps(policy_output, target_boxes, target_probs)
    res = run_bass_kernel_spmd(nc, in_maps, list(range(N_CORES)))
    total = 0.0
    for i in range(N_CORES):
        total += float(res.results[i]["out"].sum(dtype=np.float64))
    return np.float32(total / DENOM)


# revision 29
# speedup vs baseline: 1.0810x; 1.0810x over previous
"""Trainium2 Bass kernel for nn_BoxDetectionLoss (8-core data parallel).

Math: reference loss = sum_{a,r,c}[ has_match ? coord+conf_loss : conf^2 ] / denom.
A pixel (r,c) can only match a target box t if r==tb[t,0] and c==tb[t,1]
(T=16 boxes per image), so the dense term is just sum sigmoid(conf_ch)^2 over
channels {2,5,8}; the match term is a correction at <=16 pixels x 3 anchors,
computed from 144 gathered elements per image.

Each of the 8 cores handles one batch image:
  - dense: 6 chunk DMAs of [128,1024] f32 spread over the three DMA rings
    (sync/scalar HWDGE + gpsimd SWDGE) so compute pipelines with the loads;
    the shared-tag bufs=4 pool throttles DMAs in flight, which staggers ring
    contention. Per chunk: ACT sigmoid (bf16 out); the square+reduce runs as
    DVE bf16 tensor_tensor mult + PE matmul against a ones vector that
    accumulates column sums into one [1,512] PSUM bank (the idle PE replaces
    DVE's slow 1x-mode tensor_reduce); the last-arriving chunk instead uses
    ACT Square w/ accum_out so the final PSUM fold stays off the tail.
  - correction: host precomputes all tb-derived constants (r1,c1,r2,c2,tp,
    keep-mask, gather offsets) into one packed [16,15] i32 tensor; device
    gathers pol at the 16 box pixels x 9 channels, applies sigmoid, and ~19
    tiny [16,3] DVE ops produce the correction column - all overlapped with
    the dense DMA window.
  - output: ACC [128, 7] partials DMA'd out; host sums and divides by denom.

Only standard-ISA engine ops are used: custom DVE ops (tensor_tensor_reduce
etc.) crash this runtime's exec units (no custom ucode tables loaded).
"""

import numpy as np

B, C, H, W = 8, 9, 512, 512
T = 16
N_CORES = 8
CONF_CH = (2, 5, 8)
DENOM = float(B * H * W * 3)
MAGIC = 12582912.0  # 1.5 * 2^23: x+MAGIC-MAGIC rounds to nearest-even int
NCHUNK = 6
NCOL = NCHUNK + 1  # dense chunk columns + correction column

# Dense chunk schedule: channel, column window of the [128,2048] channel view,
# DMA queue, square engine (act=True -> ACT Square+accum, else DVE TT+TR).
# One channel per queue, split big-first/small-second: the big first wave
# starts compute early; the small second wave keeps the post-DMA tail short.
CHUNK_SPEC = (
    dict(ch=2, lo=0, n=1024, q="sync", act=False),      # c0
    dict(ch=5, lo=0, n=1024, q="scalar", act=False),    # c1
    dict(ch=8, lo=0, n=1024, q="gpsimd", act=False),    # c2
    dict(ch=2, lo=1024, n=1024, q="sync", act=False),   # c3
    dict(ch=5, lo=1024, n=1024, q="scalar", act=False),  # c4
    dict(ch=8, lo=1024, n=1024, q="gpsimd", act=True),  # c5
)
ACT_ORDER = (0, 1, 2, 3, 4, 5)  # expected arrival order for the compute loop
DVE_SQ_ORDER = (0, 1, 2, 3)  # DVE-square chunks in expected arrival order

_PROG = None


def _emit_correction(nc, sp, ACC, bass, mybir, GS, CF, col):
    """~19 tiny [16,3] DVE ops: prediction, rounding, match test, loss terms.

    GS: [T, C] f32 sigmoid of gathered pol values at box pixels.
    CF: [T, 15] f32 view of packed constants; cols 0-5 = r1,c1,r2,c2,tp,keep.
    """
    f32 = mybir.dt.float32
    ALU = mybir.AluOpType
    R1, C1 = CF[:, 0:1], CF[:, 1:2]
    R2, C2 = CF[:, 2:3], CF[:, 3:4]
    TP, KEEP = CF[:, 4:5], CF[:, 5:6]

    # channel ch = 3a + k: k=0 delta_r, k=1 delta_c, k=2 conf
    gs3 = GS[:].rearrange("p (a k) -> p k a", k=3)

    predr = sp.tile([T, 3], f32)
    nc.vector.tensor_scalar(out=predr[:], in0=gs3[:, 0, :], scalar1=9.0,
                            scalar2=R1, op0=ALU.mult, op1=ALU.add)
    nc.vector.tensor_scalar(out=predr[:], in0=predr[:], scalar1=511.0,
                            scalar2=0.0, op0=ALU.min, op1=ALU.max)
    predc = sp.tile([T, 3], f32)
    nc.vector.tensor_scalar(out=predc[:], in0=gs3[:, 1, :], scalar1=16.0,
                            scalar2=C1, op0=ALU.mult, op1=ALU.add)
    nc.vector.tensor_scalar(out=predc[:], in0=predc[:], scalar1=511.0,
                            scalar2=0.0, op0=ALU.min, op1=ALU.max)

    # round-to-nearest-even via the +/- 1.5*2^23 trick, fused in one op
    rr = sp.tile([T, 3], f32)
    nc.vector.tensor_scalar(out=rr[:], in0=predr[:], scalar1=MAGIC,
                            scalar2=MAGIC, op0=ALU.add, op1=ALU.subtract)
    rc = sp.tile([T, 3], f32)
    nc.vector.tensor_scalar(out=rc[:], in0=predc[:], scalar1=MAGIC,
                            scalar2=MAGIC, op0=ALU.add, op1=ALU.subtract)

    # match mask; keep folds out duplicate boxes (first-match semantics)
    m = sp.tile([T, 3], f32)
    nc.vector.tensor_scalar(out=m[:], in0=rr[:], scalar1=R2, scalar2=None,
                            op0=ALU.is_equal)
    m2 = sp.tile([T, 3], f32)
    nc.vector.tensor_scalar(out=m2[:], in0=rc[:], scalar1=C2, scalar2=None,
                            op0=ALU.is_equal)
    nc.vector.tensor_tensor(out=m[:], in0=m[:], in1=m2[:], op=ALU.mult)
    nc.vector.tensor_scalar(out=m[:], in0=m[:], scalar1=KEEP, scalar2=None,
                            op0=ALU.mult)

    # contribution = |predr-r2| + |predc-c2| + tp*(tp-2*conf)
    # |x| as max(predr-r2, r2-predr): abs is not in the DVE TS ISA
    d1 = sp.tile([T, 3], f32)
    nc.vector.tensor_scalar(out=d1[:], in0=predr[:], scalar1=R2, scalar2=None,
                            op0=ALU.subtract)
    d1n = sp.tile([T, 3], f32)
    nc.vector.tensor_scalar(out=d1n[:], in0=predr[:], scalar1=-1.0, scalar2=R2,
                            op0=ALU.mult, op1=ALU.add)
    nc.vector.tensor_tensor(out=d1[:], in0=d1[:], in1=d1n[:], op=ALU.max)
    d2 = sp.tile([T, 3], f32)
    nc.vector.tensor_scalar(out=d2[:], in0=predc[:], scalar1=C2, scalar2=None,
                            op0=ALU.subtract)
    d2n = sp.tile([T, 3], f32)
    nc.vector.tensor_scalar(out=d2n[:], in0=predc[:], scalar1=-1.0, scalar2=C2,
                            op0=ALU.mult, op1=ALU.add)
    nc.vector.tensor_tensor(out=d2[:], in0=d2[:], in1=d2n[:], op=ALU.max)
    nc.vector.tensor_tensor(out=d1[:], in0=d1[:], in1=d2[:], op=ALU.add)
    cf = sp.tile([T, 3], f32)
    nc.vector.tensor_scalar(out=cf[:], in0=gs3[:, 2, :], scalar1=-2.0,
                            scalar2=TP, op0=ALU.mult, op1=ALU.add)
    nc.vector.tensor_scalar(out=cf[:], in0=cf[:], scalar1=TP, scalar2=None,
                            op0=ALU.mult)
    nc.vector.tensor_tensor(out=d1[:], in0=d1[:], in1=cf[:], op=ALU.add)

    # ACC[0:T, col] = sum_anchors m * d1
    nc.vector.tensor_tensor(out=m[:], in0=m[:], in1=d1[:], op=ALU.mult)
    nc.vector.tensor_reduce(out=ACC[0:T, col:col + 1], in_=m[:],
                            axis=mybir.AxisListType.X, op=ALU.add)


def _build_program(chunk_spec=CHUNK_SPEC, act_order=ACT_ORDER,
                   dve_sq_order=DVE_SQ_ORDER, corr=True, gather=True):
    import concourse.bass as bass
    import concourse.tile as tile
    from concourse import bacc, mybir

    f32 = mybir.dt.float32
    i32 = mybir.dt.int32
    bf16 = mybir.dt.bfloat16
    ALU = mybir.AluOpType
    ACT_F = mybir.ActivationFunctionType
    nchunk = len(chunk_spec)

    nc = bacc.Bacc(
        "TRN2", target_bir_lowering=False, debug=False, num_devices=N_CORES
    )
    pol = nc.dram_tensor("pol", [C, H, W], f32, kind="ExternalInput").ap()
    cst = nc.dram_tensor("cst", [T, 15], i32, kind="ExternalInput").ap()
    out = nc.dram_tensor("out", [128, nchunk + 1], f32,
                         kind="ExternalOutput").ap()

    with tile.TileContext(nc) as tc:
        with (
            tc.tile_pool(name="io", bufs=4) as io,
            tc.tile_pool(name="acc", bufs=1) as accp,
            tc.tile_pool(name="small", bufs=1) as sp,
            tc.tile_pool(name="psum", bufs=1, space="PSUM") as psp,
        ):
            ACC = accp.tile([128, nchunk + 1], f32)
            nc.vector.memset(ACC[:], 0.0)
            # ones vector: PE matmul against it sums SQ over partitions into
            # one accumulating PSUM bank (frees DVE from 1x-mode reduces)
            ONES = sp.tile([128, 1], bf16)
            nc.vector.memset(ONES[:], 1.0)
            PS = psp.tile([1, 512], f32, space="PSUM")

            # correction inputs: packed constants + indirect gather of the
            # 16 box pixels x 9 channels (offsets precomputed on host)
            CST = sp.tile([T, 15], i32)
            G = sp.tile([T, C], f32)
            if corr:
                nc.scalar.dma_start(CST[:], cst[:])
            xts = []
            for k, cs in enumerate(chunk_spec):
                view = pol[cs["ch"]].rearrange("(p a) w -> p (a w)", p=128)
                Xt = io.tile([128, cs["n"]], f32, tag="in")
                getattr(nc, cs["q"]).dma_start(
                    Xt[:], view[:, cs["lo"]:cs["lo"] + cs["n"]])
                xts.append(Xt)

            if corr and gather:
                # dense offset tile for the SWDGE offset walker
                OFFD = sp.tile([T, C], i32)
                nc.vector.tensor_copy(OFFD[:], CST[:, 6:15])
                nc.gpsimd.indirect_dma_start(
                    out=G[:], out_offset=None,
                    in_=pol.rearrange("c h (w a) -> (c h w) a", a=1),
                    in_offset=bass.IndirectOffsetOnAxis(ap=OFFD[:], axis=0),
                )
            elif corr:
                nc.vector.memset(G[:], 0.0)

            GS = sp.tile([T, C], f32)
            CF = CST[:].bitcast(f32)
            dve_ks = [k for k in act_order if not chunk_spec[k]["act"]]
            for k in act_order:
                cs, Xt = chunk_spec[k], xts[k]
                SIG = io.tile([128, cs["n"]], bf16, tag="sig")
                nc.scalar.activation(SIG[:], Xt[:], ACT_F.Sigmoid)
                if corr and k == act_order[0]:
                    nc.scalar.activation(GS[:], G[:], ACT_F.Sigmoid)
                SQ = io.tile([128, cs["n"]], bf16, tag="sq")
                if cs["act"]:
                    nc.scalar.activation(SQ[:], SIG[:], ACT_F.Square,
                                         accum_out=ACC[:, k:k + 1])
                else:
                    nc.vector.tensor_tensor(out=SQ[:], in0=SIG[:], in1=SIG[:],
                                            op=ALU.mult)
                    for blk in range(0, cs["n"], 512):
                        nc.tensor.matmul(
                            out=PS[:], lhsT=ONES[:],
                            rhs=SQ[:, blk:blk + 512],
                            start=(k == dve_ks[0] and blk == 0),
                            stop=(k == dve_ks[-1] and blk + 512 >= cs["n"]),
                        )
                if corr and k == act_order[1]:
                    _emit_correction(nc, sp, ACC, bass, mybir, GS, CF, nchunk)

            # fold the PE-accumulated column sums into ACC[0,0]
            nc.vector.tensor_reduce(out=ACC[0:1, 0:1], in_=PS[:],
                                    axis=mybir.AxisListType.X, op=ALU.add)
            nc.sync.dma_start(out[:], ACC[:])

    nc.compile()
    return nc


def get_program():
    global _PROG
    if _PROG is None:
        _PROG = _build_program()
    return _PROG


def make_in_maps(policy_output, target_boxes, target_probs):
    policy_output = np.ascontiguousarray(np.asarray(policy_output, dtype=np.float32))
    target_boxes = np.ascontiguousarray(np.asarray(target_boxes, dtype=np.int32))
    target_probs = np.ascontiguousarray(np.asarray(target_probs, dtype=np.float32))
    assert policy_output.shape == (B, C, H, W)
    in_maps = []
    for i in range(N_CORES):
        tb = target_boxes[i].astype(np.int64)
        r1, c1, r2, c2 = tb[:, 0], tb[:, 1], tb[:, 2], tb[:, 3]
        off = (np.arange(C, dtype=np.int64)[None, :] * (H * W)
               + (r1 * W + c1)[:, None]).astype(np.int32)
        keep = np.ones(T, dtype=np.float32)
        seen = set()
        for t in range(T):
            key = (int(r1[t]), int(c1[t]), int(r2[t]), int(c2[t]))
            if key in seen:
                keep[t] = 0.0
            else:
                seen.add(key)
        cstf = np.zeros((T, 15), dtype=np.float32)
        cstf[:, 0] = r1
        cstf[:, 1] = c1
        cstf[:, 2] = r2
        cstf[:, 3] = c2
        cstf[:, 4] = target_probs[i]
        cstf[:, 5] = keep
        cst = cstf.view(np.int32).copy()
        cst[:, 6:15] = off
        in_maps.append({"pol": policy_output[i], "cst": cst})
    return in_maps


def kernel(policy_output, target_boxes, target_probs):
    from concourse.bass_utils import run_bass_kernel_spmd

    nc = get_program()
    in_maps = make_in_maps(policy_output, target_boxes, target_probs)
    res = run_bass_kernel_spmd(nc, in_maps, list(range(N_CORES)))
    total = 0.0
    for i in range(N_CORES):
        total += float(res.results[i]["out"].sum(dtype=np.float64))
    return np.float32(total / DENOM)


# revision 30
# speedup vs baseline: 1.1967x; 1.1071x over previous
"""Trainium2 Bass kernel for nn_BoxDetectionLoss (8-core data parallel).

Math: reference loss = sum_{a,r,c}[ has_match ? coord+conf_loss : conf^2 ] / denom.
A pixel (r,c) can only match a target box t if r==tb[t,0] and c==tb[t,1]
(T=16 boxes per image), so the dense term is just sum sigmoid(conf_ch)^2 over
channels {2,5,8}; the match term is a correction at <=16 pixels x 3 anchors,
computed from 144 gathered elements per image.

Each of the 8 cores handles one batch image:
  - dense: 6 chunk DMAs of [128,1024] f32 spread over the three DMA rings
    (sync/scalar HWDGE + gpsimd SWDGE) so compute pipelines with the loads;
    the shared-tag bufs=4 pool throttles DMAs in flight, which staggers ring
    contention. Per chunk: ACT sigmoid (bf16 out); the square+reduce runs as
    DVE bf16 tensor_tensor mult + PE matmul against a ones vector that
    accumulates column sums into one [1,512] PSUM bank (the idle PE replaces
    DVE's slow 1x-mode tensor_reduce); the last-arriving chunk instead uses
    ACT Square w/ accum_out so the final PSUM fold stays off the tail.
  - correction: host precomputes all tb-derived constants (r1,c1,r2,c2,tp,
    keep-mask, gather offsets) into one packed [16,15] i32 tensor; device
    gathers pol at the 16 box pixels x 9 channels, applies sigmoid, and ~19
    tiny [16,3] DVE ops produce the correction column - all overlapped with
    the dense DMA window.
  - output: ACC [128, 7] partials DMA'd out; host sums and divides by denom.

Only standard-ISA engine ops are used: custom DVE ops (tensor_tensor_reduce
etc.) crash this runtime's exec units (no custom ucode tables loaded).
"""

import numpy as np

B, C, H, W = 8, 9, 512, 512
T = 16
N_CORES = 8
CONF_CH = (2, 5, 8)
DENOM = float(B * H * W * 3)
MAGIC = 12582912.0  # 1.5 * 2^23: x+MAGIC-MAGIC rounds to nearest-even int
NCHUNK = 6
NCOL = NCHUNK + 1  # dense chunk columns + correction column

# Dense chunk schedule: channel, column window of the [128,2048] channel view,
# DMA queue, square engine (act=True -> ACT Square+accum, else DVE TT+TR).
# One channel per queue, split big-first/small-second: the big first wave
# starts compute early; the small second wave keeps the post-DMA tail short.
CHUNK_SPEC = (
    dict(ch=2, lo=0, n=1024, q="sync", act=False),      # c0
    dict(ch=5, lo=0, n=1024, q="scalar", act=False),    # c1
    dict(ch=8, lo=0, n=1024, q="gpsimd", act=False),    # c2
    dict(ch=2, lo=1024, n=1024, q="sync", act=False),   # c3
    dict(ch=5, lo=1024, n=1024, q="scalar", act=False),  # c4
    dict(ch=8, lo=1024, n=1024, q="gpsimd", act=True),  # c5
)
ACT_ORDER = (0, 1, 2, 3, 4, 5)  # expected arrival order for the compute loop
DVE_SQ_ORDER = (0, 1, 2, 3)  # DVE-square chunks in expected arrival order

_PROG = None


def _emit_correction(nc, sp, ACC, bass, mybir, GS, CF, col):
    """~19 tiny [16,3] DVE ops: prediction, rounding, match test, loss terms.

    GS: [T, C] f32 sigmoid of gathered pol values at box pixels.
    CF: [T, 15] f32 view of packed constants; cols 0-5 = r1,c1,r2,c2,tp,keep.
    """
    f32 = mybir.dt.float32
    ALU = mybir.AluOpType
    R1, C1 = CF[:, 0:1], CF[:, 1:2]
    R2, C2 = CF[:, 2:3], CF[:, 3:4]
    TP, KEEP = CF[:, 4:5], CF[:, 5:6]

    # channel ch = 3a + k: k=0 delta_r, k=1 delta_c, k=2 conf
    gs3 = GS[:].rearrange("p (a k) -> p k a", k=3)

    predr = sp.tile([T, 3], f32)
    nc.vector.tensor_scalar(out=predr[:], in0=gs3[:, 0, :], scalar1=9.0,
                            scalar2=R1, op0=ALU.mult, op1=ALU.add)
    nc.vector.tensor_scalar(out=predr[:], in0=predr[:], scalar1=511.0,
                            scalar2=0.0, op0=ALU.min, op1=ALU.max)
    predc = sp.tile([T, 3], f32)
    nc.vector.tensor_scalar(out=predc[:], in0=gs3[:, 1, :], scalar1=16.0,
                            scalar2=C1, op0=ALU.mult, op1=ALU.add)
    nc.vector.tensor_scalar(out=predc[:], in0=predc[:], scalar1=511.0,
                            scalar2=0.0, op0=ALU.min, op1=ALU.max)

    # round-to-nearest-even via the +/- 1.5*2^23 trick, fused in one op
    rr = sp.tile([T, 3], f32)
    nc.vector.tensor_scalar(out=rr[:], in0=predr[:], scalar1=MAGIC,
                            scalar2=MAGIC, op0=ALU.add, op1=ALU.subtract)
    rc = sp.tile([T, 3], f32)
    nc.vector.tensor_scalar(out=rc[:], in0=predc[:], scalar1=MAGIC,
                            scalar2=MAGIC, op0=ALU.add, op1=ALU.subtract)

    # match mask; keep folds out duplicate boxes (first-match semantics)
    m = sp.tile([T, 3], f32)
    nc.vector.tensor_scalar(out=m[:], in0=rr[:], scalar1=R2, scalar2=None,
                            op0=ALU.is_equal)
    m2 = sp.tile([T, 3], f32)
    nc.vector.tensor_scalar(out=m2[:], in0=rc[:], scalar1=C2, scalar2=None,
                            op0=ALU.is_equal)
    nc.vector.tensor_tensor(out=m[:], in0=m[:], in1=m2[:], op=ALU.mult)
    nc.vector.tensor_scalar(out=m[:], in0=m[:], scalar1=KEEP, scalar2=None,
                            op0=ALU.mult)

    # contribution = |predr-r2| + |predc-c2| + tp*(tp-2*conf)
    # |x| as max(predr-r2, r2-predr): abs is not in the DVE TS ISA
    d1 = sp.tile([T, 3], f32)
    nc.vector.tensor_scalar(out=d1[:], in0=predr[:], scalar1=R2, scalar2=None,
                            op0=ALU.subtract)
    d1n = sp.tile([T, 3], f32)
    nc.vector.tensor_scalar(out=d1n[:], in0=predr[:], scalar1=-1.0, scalar2=R2,
                            op0=ALU.mult, op1=ALU.add)
    nc.vector.tensor_tensor(out=d1[:], in0=d1[:], in1=d1n[:], op=ALU.max)
    d2 = sp.tile([T, 3], f32)
    nc.vector.tensor_scalar(out=d2[:], in0=predc[:], scalar1=C2, scalar2=None,
                            op0=ALU.subtract)
    d2n = sp.tile([T, 3], f32)
    nc.vector.tensor_scalar(out=d2n[:], in0=predc[:], scalar1=-1.0, scalar2=C2,
                            op0=ALU.mult, op1=ALU.add)
    nc.vector.tensor_tensor(out=d2[:], in0=d2[:], in1=d2n[:], op=ALU.max)
    nc.vector.tensor_tensor(out=d1[:], in0=d1[:], in1=d2[:], op=ALU.add)
    cf = sp.tile([T, 3], f32)
    nc.vector.tensor_scalar(out=cf[:], in0=gs3[:, 2, :], scalar1=-2.0,
                            scalar2=TP, op0=ALU.mult, op1=ALU.add)
    nc.vector.tensor_scalar(out=cf[:], in0=cf[:], scalar1=TP, scalar2=None,
                            op0=ALU.mult)
    nc.vector.tensor_tensor(out=d1[:], in0=d1[:], in1=cf[:], op=ALU.add)

    # ACC[0:T, col] = sum_anchors m * d1
    nc.vector.tensor_tensor(out=m[:], in0=m[:], in1=d1[:], op=ALU.mult)
    nc.vector.tensor_reduce(out=ACC[0:T, col:col + 1], in_=m[:],
                            axis=mybir.AxisListType.X, op=ALU.add)


def _build_program(chunk_spec=CHUNK_SPEC, act_order=ACT_ORDER,
                   dve_sq_order=DVE_SQ_ORDER, corr=True, gather=True):
    import concourse.bass as bass
    import concourse.tile as tile
    from concourse import bacc, mybir

    f32 = mybir.dt.float32
    i32 = mybir.dt.int32
    bf16 = mybir.dt.bfloat16
    ALU = mybir.AluOpType
    ACT_F = mybir.ActivationFunctionType
    nchunk = len(chunk_spec)

    nc = bacc.Bacc(
        "TRN2", target_bir_lowering=False, debug=False, num_devices=N_CORES
    )
    pol = nc.dram_tensor("pol", [C, H, W], f32, kind="ExternalInput").ap()
    cst = nc.dram_tensor("cst", [T, 15], i32, kind="ExternalInput").ap()
    out = nc.dram_tensor("out", [128, nchunk + 1], f32,
                         kind="ExternalOutput").ap()

    with tile.TileContext(nc) as tc:
        with (
            tc.tile_pool(name="io", bufs=5) as io,
            tc.tile_pool(name="acc", bufs=1) as accp,
            tc.tile_pool(name="small", bufs=1) as sp,
            tc.tile_pool(name="psum", bufs=1, space="PSUM") as psp,
        ):
            ACC = accp.tile([128, nchunk + 1], f32)
            nc.vector.memset(ACC[:], 0.0)
            # ones vector: PE matmul against it sums SQ over partitions into
            # one accumulating PSUM bank (frees DVE from 1x-mode reduces)
            ONES = sp.tile([128, 1], bf16)
            nc.vector.memset(ONES[:], 1.0)
            PS = psp.tile([1, 512], f32, space="PSUM")

            # correction inputs: packed constants + indirect gather of the
            # 16 box pixels x 9 channels (offsets precomputed on host)
            CST = sp.tile([T, 15], i32)
            G = sp.tile([T, C], f32)
            if corr:
                nc.scalar.dma_start(CST[:], cst[:])
            xts = []
            for k, cs in enumerate(chunk_spec):
                view = pol[cs["ch"]].rearrange("(p a) w -> p (a w)", p=128)
                Xt = io.tile([128, cs["n"]], f32, tag="in")
                getattr(nc, cs["q"]).dma_start(
                    Xt[:], view[:, cs["lo"]:cs["lo"] + cs["n"]])
                xts.append(Xt)

            if corr and gather:
                # dense offset tile for the SWDGE offset walker
                OFFD = sp.tile([T, C], i32)
                nc.vector.tensor_copy(OFFD[:], CST[:, 6:15])
                nc.gpsimd.indirect_dma_start(
                    out=G[:], out_offset=None,
                    in_=pol.rearrange("c h (w a) -> (c h w) a", a=1),
                    in_offset=bass.IndirectOffsetOnAxis(ap=OFFD[:], axis=0),
                )
            elif corr:
                nc.vector.memset(G[:], 0.0)

            GS = sp.tile([T, C], f32)
            CF = CST[:].bitcast(f32)
            dve_ks = [k for k in act_order if not chunk_spec[k]["act"]]
            for k in act_order:
                cs, Xt = chunk_spec[k], xts[k]
                SIG = io.tile([128, cs["n"]], bf16, tag="sig")
                nc.scalar.activation(SIG[:], Xt[:], ACT_F.Sigmoid)
                if corr and k == act_order[0]:
                    nc.scalar.activation(GS[:], G[:], ACT_F.Sigmoid)
                SQ = io.tile([128, cs["n"]], bf16, tag="sq")
                if cs["act"]:
                    nc.scalar.activation(SQ[:], SIG[:], ACT_F.Square,
                                         accum_out=ACC[:, k:k + 1])
                else:
                    nc.vector.tensor_tensor(out=SQ[:], in0=SIG[:], in1=SIG[:],
                                            op=ALU.mult)
                    for blk in range(0, cs["n"], 512):
                        nc.tensor.matmul(
                            out=PS[:], lhsT=ONES[:],
                            rhs=SQ[:, blk:blk + 512],
                            start=(k == dve_ks[0] and blk == 0),
                            stop=(k == dve_ks[-1] and blk + 512 >= cs["n"]),
                        )
                if corr and k == act_order[1]:
                    _emit_correction(nc, sp, ACC, bass, mybir, GS, CF, nchunk)

            # fold the PE-accumulated column sums into ACC[0,0]
            nc.vector.tensor_reduce(out=ACC[0:1, 0:1], in_=PS[:],
                                    axis=mybir.AxisListType.X, op=ALU.add)
            nc.sync.dma_start(out[:], ACC[:])

    nc.compile()
    return nc


def get_program():
    global _PROG
    if _PROG is None:
        _PROG = _build_program()
    return _PROG


def make_in_maps(policy_output, target_boxes, target_probs):
    policy_output = np.ascontiguousarray(np.asarray(policy_output, dtype=np.float32))
    target_boxes = np.ascontiguousarray(np.asarray(target_boxes, dtype=np.int32))
    target_probs = np.ascontiguousarray(np.asarray(target_probs, dtype=np.float32))
    assert policy_output.shape == (B, C, H, W)
    in_maps = []
    for i in range(N_CORES):
        tb = target_boxes[i].astype(np.int64)
        r1, c1, r2, c2 = tb[:, 0], tb[:, 1], tb[:, 2], tb[:, 3]
        off = (np.arange(C, dtype=np.int64)[None, :] * (H * W)
               + (r1 * W + c1)[:, None]).astype(np.int32)
        keep = np.ones(T, dtype=np.float32)
        seen = set()
        for t in range(T):
            key = (int(r1[t]), int(c1[t]), int(r2[t]), int(c2[t]))
            if key in seen:
                keep[t] = 0.0
            else:
                seen.add(key)
        cstf = np.zeros((T, 15), dtype=np.float32)
        cstf[:, 0] = r1
        cstf[:, 1] = c1
        cstf[:, 2] = r2
        cstf[:, 3] = c2
        cstf[:, 4] = target_probs[i]
        cstf[:, 5] = keep
        cst = cstf.view(np.int32).copy()
        cst[:, 6:15] = off
        in_maps.append({"pol": policy_output[i], "cst": cst})
    return in_maps


def kernel(policy_output, target_boxes, target_probs):
    from concourse.bass_utils import run_bass_kernel_spmd

    nc = get_program()
    in_maps = make_in_maps(policy_output, target_boxes, target_probs)
    res = run_bass_kernel_spmd(nc, in_maps, list(range(N_CORES)))
    total = 0.0
    for i in range(N_CORES):
        total += float(res.results[i]["out"].sum(dtype=np.float64))
    return np.float32(total / DENOM)
